# revision 20
# baseline (speedup 1.0000x reference)
"""TBCNN conv-node kernel for Trainium2 (8 NeuronCores, batch-sharded).

Math (derived from the reference, including its faithful-reshape quirk):
  out[b,n,o] = tanh( nodes[b,n,:] @ Wt + Sr[b,n,:] @ Wr + Sl[b,n,:] @ Wl + bias[o] )
    Sr[b,n,:] = sum_c cr[b,n,c] * nodes[b, ch[b,n,c], :]
    Sl[b,n,:] = sum_c cl[b,n,c] * nodes[b, ch[b,n,c], :]
  where Wt/Wr/Wl are rows 0::3 / 1::3 / 2::3 of concat([w_t, w_r, w_l]) (the
  reference reshapes [F,3] -> [3,F] raw), and cr/cl are the eta_r/eta_l
  coefficients, both forced to 0 where ch==0 so the zero-row lookup semantics
  hold while gathering from the raw nodes table.

v2 layout: the gather/stage-1 path is bf16; stage 2 stays f32r (same PE
speed at 512-col free dim, and bf16 weights alone would cost 0.8% error):
  - nodes gathered in bf16 (half the SWDGE DMA bytes),
  - stage-1 block matmuls bf16 (single-issue on PE, half the LDWEIGHTS bytes;
    fp32 matmuls double-pump),
  - host supplies nodesT / per-block coefficient matrices / de-interleaved
    weights directly, so no on-device transposes or coefficient DVE chain
    remain,
  - output written o-major ([O, N] per batch) and transposed on host.

Per core (2 batches):
  - dma_gather (4 SWDGE queues round-robin, 1024 bf16 rows per call) fetches
    child rows in a (node-octet, child) x feature partition layout.
  - Per 8-node block, one bf16 PE matmul against a [128,16] block-diagonal
    coefficient matrix (8 eta_r + 8 eta_l columns) reduces children into
    feature-major SrT/SlT.
  - Stage 2 per 512 nodes: 3 accumulated bf16 matmuls (parent/right/left)
    + bias add + tanh, interleaved with the gather chunks.
"""

import numpy as np
from functools import lru_cache

B, N, C, F, O = 16, 2048, 16, 128, 128
NCORES = 8
BPC = B // NCORES  # batches per core
KBLK = 8  # 8-node-block gather groups per chunk (KBLK*128 rows per dma_gather)
NBLK = N // 8  # 256 blocks per batch
NCHUNK = NBLK // KBLK  # 32 chunks per batch
NPC = KBLK * 8  # nodes covered per chunk (64)

f16_np = np.float16


@lru_cache(maxsize=1)
def _build():
    import concourse.bass as bass
    import concourse.bacc as bacc
    import concourse.tile as tile
    from concourse import mybir

    f32 = mybir.dt.float32
    f32r = mybir.dt.float32r
    f16 = mybir.dt.float16
    i32 = mybir.dt.int32
    Act = mybir.ActivationFunctionType

    nc = bacc.Bacc("TRN2", target_bir_lowering=False, debug=False,
                   num_devices=NCORES, num_swdge_queues=4)

    nodesb_d = nc.dram_tensor("nodesb", [BPC * N, F], f16, kind="ExternalInput")
    nodest_d = nc.dram_tensor("nodest", [BPC, 128, N], f32, kind="ExternalInput")
    idxt_d = nc.dram_tensor("idxt", [BPC, 128, NBLK], i32, kind="ExternalInput")
    acoef_d = nc.dram_tensor("acoef", [BPC, 128, NBLK * 16], f16,
                             kind="ExternalInput")
    wt_d = nc.dram_tensor("wt2", [F, O], f32, kind="ExternalInput")
    wr_d = nc.dram_tensor("wr2", [F, O], f32, kind="ExternalInput")
    wl_d = nc.dram_tensor("wl2", [F, O], f32, kind="ExternalInput")
    bc_d = nc.dram_tensor("bcol", [128, 1], f32, kind="ExternalInput")
    outt_d = nc.dram_tensor("outt", [BPC, 128, N], f32, kind="ExternalOutput")

    with tile.TileContext(nc) as tc:
        with (
            tc.tile_pool(name="const", bufs=1) as cpool,
            tc.tile_pool(name="work", bufs=2) as pool,
            tc.tile_pool(name="gath", bufs=16) as gpool,
            tc.tile_pool(name="perb", bufs=2) as ppool,
            tc.tile_pool(name="ps1", bufs=3, space="PSUM") as ps1pool,
            tc.tile_pool(name="ps2", bufs=2, space="PSUM") as ps2pool,
        ):
            # ---------------- constants ----------------
            wt_s = cpool.tile([F, O], f32)
            wr_s = cpool.tile([F, O], f32)
            wl_s = cpool.tile([F, O], f32)
            bc_s = cpool.tile([128, 1], f32)
            nc.sync.dma_start(wt_s[:], wt_d.ap())
            nc.sync.dma_start(wr_s[:], wr_d.ap())
            nc.sync.dma_start(wl_s[:], wl_d.ap())
            nc.sync.dma_start(bc_s[:], bc_d.ap())
            wtr_s = cpool.tile([F, O], f32r)
            wrr_s = cpool.tile([F, O], f32r)
            wlr_s = cpool.tile([F, O], f32r)
            nc.vector.tensor_copy(wtr_s[:], wt_s[:])
            nc.vector.tensor_copy(wrr_s[:], wr_s[:])
            nc.vector.tensor_copy(wlr_s[:], wl_s[:])

            for b in range(BPC):
                # ------------- per-batch staged inputs -------------
                idxt = ppool.tile([128, NBLK], i32)
                nc.sync.dma_start(idxt[:], idxt_d.ap()[b])
                acoef = ppool.tile([128, NBLK * 16], f16)
                nc.sync.dma_start(acoef[:], acoef_d.ap()[b])
                nodestf = ppool.tile([128, N], f32)
                nc.sync.dma_start(nodestf[:], nodest_d.ap()[b])
                nodest = ppool.tile([128, N], f32r)
                nc.vector.tensor_copy(nodest[:], nodestf[:])

                # ------------- gather + stage 1 + interleaved stage 2 -------
                srt = ppool.tile([128, N], f32r)
                slt = ppool.tile([128, N], f32r)
                for q in range(NCHUNK):
                    g = gpool.tile([128, KBLK * 128], f16)
                    for gl in range(KBLK):
                        nc.gpsimd.indirect_dma_start(
                            out=g[:, gl * 128:(gl + 1) * 128],
                            out_offset=None,
                            in_=nodesb_d.ap(),
                            in_offset=bass.IndirectOffsetOnAxis(
                                ap=idxt[:, q * KBLK + gl:q * KBLK + gl + 1],
                                axis=0,
                            ),
                        )
                    ps1 = ps1pool.tile([128, KBLK * 16], f32)
                    for gl in range(KBLK):
                        nc.tensor.matmul(
                            ps1[:, gl * 16:(gl + 1) * 16],
                            lhsT=g[:, gl * 128:(gl + 1) * 128],
                            rhs=acoef[:, (q * KBLK + gl) * 16:
                                      (q * KBLK + gl + 1) * 16],
                            start=True,
                            stop=True,
                        )
                    nc.vector.tensor_copy(
                        srt[:, q * NPC:(q + 1) * NPC].rearrange(
                            "p (g m) -> p g m", m=8
                        ),
                        ps1[:].rearrange("p (g m) -> p g m", m=16)[:, :, 0:8],
                    )
                    nc.vector.tensor_copy(
                        slt[:, q * NPC:(q + 1) * NPC].rearrange(
                            "p (g m) -> p g m", m=8
                        ),
                        ps1[:].rearrange("p (g m) -> p g m", m=16)[:, :, 8:16],
                    )

                    # ---- stage 2 for the 512 nodes completed by this chunk
                    if q % 8 == 7:
                        rnd = q // 8
                        sl = slice(rnd * 512, (rnd + 1) * 512)
                        ps2 = ps2pool.tile([128, 512], f32)
                        nc.tensor.matmul(
                            ps2[:], lhsT=wtr_s[:], rhs=nodest[:, sl],
                            start=True, stop=False,
                        )
                        nc.tensor.matmul(
                            ps2[:], lhsT=wrr_s[:], rhs=srt[:, sl],
                            start=False, stop=False,
                        )
                        nc.tensor.matmul(
                            ps2[:], lhsT=wlr_s[:], rhs=slt[:, sl],
                            start=False, stop=True,
                        )
                        ot = pool.tile([128, 512], f32)
                        nc.scalar.activation(ot[:], ps2[:], Act.Tanh, bias=bc_s[:])
                        nc.sync.dma_start(outt_d.ap()[b][:, sl], ot[:])

    nc.compile()
    return nc


def _coeffs(ch):
    """eta_r / eta_l effective coefficients per (node, child-slot).

    Faithful to the reference: for num_siblings==1 the 0.5 goes to slot 0
    regardless of where the single real child sits.
    """
    chf = (ch != 0)
    mk = chf.astype(np.float32)  # [*, C]
    K = mk.sum(-1, keepdims=True)
    cidx = np.arange(C, dtype=np.float32)
    denom = K - 1.0
    safe = np.where(denom == 0.0, 1.0, denom)
    cr_multi = cidx * mk / safe
    single = np.zeros((C,), np.float32)
    single[0] = 0.5
    cr = np.where(K == 1.0, single, cr_multi)
    creff = cr * mk
    cleff = mk - creff
    return creff, cleff


def _host_prep(nodes, children, w_t, w_r, w_l, b_conv):
    nodes = np.ascontiguousarray(np.asarray(nodes, dtype=np.float32))
    children = np.ascontiguousarray(np.asarray(children, dtype=np.int32))
    w_t = np.asarray(w_t, dtype=np.float32)
    w_r = np.asarray(w_r, dtype=np.float32)
    w_l = np.asarray(w_l, dtype=np.float32)
    b_conv = np.asarray(b_conv, dtype=np.float32)

    wflat = np.concatenate([w_t, w_r, w_l], axis=0)  # [3F, O]
    wt2 = np.ascontiguousarray(wflat[0::3])
    wr2 = np.ascontiguousarray(wflat[1::3])
    wl2 = np.ascontiguousarray(wflat[2::3])
    bcol = np.ascontiguousarray(b_conv[:, None])  # [128, 1]

    nodesb = nodes.astype(f16_np)  # [B, N, F]
    nodest = np.ascontiguousarray(nodes.transpose(0, 2, 1))  # [B, 128, N] f32

    creff, cleff = _coeffs(children)  # [B, N, C] each

    # acoef[b, o*16+c, nb*16 + j*8 + m] = (o==m) * coef_j[b, nb*8+o, c]
    acoef = np.zeros((B, 128, NBLK, 2, 8), np.float32)
    for j, coef in ((0, creff), (1, cleff)):
        cb = coef.reshape(B, NBLK, 8, C).transpose(0, 2, 3, 1)  # [B, o, c, nb]
        for o in range(8):
            acoef[:, o * 16:(o + 1) * 16, :, j, o] = cb[:, o]
    acoef = np.ascontiguousarray(
        acoef.reshape(B, 128, NBLK * 16).astype(f16_np)
    )

    # idxt[b, o*16+c, nb] = ch[b, nb*8+o, c] + b*N  (flat row index into the
    # per-core [BPC*N, F] gather table)
    idxt = children.reshape(B, NBLK, 8, C).transpose(0, 2, 3, 1).reshape(
        B, 128, NBLK
    ).astype(np.int32)

    in_maps = []
    for core in range(NCORES):
        bs = slice(core * BPC, (core + 1) * BPC)
        idxc = idxt[bs] + (np.arange(BPC, dtype=np.int32) * N)[:, None, None]
        in_maps.append(
            {
                "nodesb": np.ascontiguousarray(nodesb[bs].reshape(BPC * N, F)),
                "nodest": np.ascontiguousarray(nodest[bs]),
                "idxt": np.ascontiguousarray(idxc),
                "acoef": np.ascontiguousarray(acoef[bs]),
                "wt2": wt2,
                "wr2": wr2,
                "wl2": wl2,
                "bcol": bcol,
            }
        )
    return in_maps


def _run(inputs, trace=False):
    from concourse.bass_utils import run_bass_kernel_spmd

    nc = _build()
    in_maps = _host_prep(
        inputs["nodes"], inputs["children"], inputs["w_t"], inputs["w_r"],
        inputs["w_l"], inputs["b_conv"],
    )
    res = run_bass_kernel_spmd(nc, in_maps, list(range(NCORES)), trace=trace)
    outt = np.concatenate([r["outt"] for r in res.results], axis=0)  # [B,128,N]
    out = np.ascontiguousarray(outt.transpose(0, 2, 1))  # [B, N, O]
    return out.astype(np.float32), res


def kernel(nodes, children, feature_size=None, w_t=None, w_r=None, w_l=None,
           b_conv=None, **_unused):
    out, _ = _run(
        {
            "nodes": nodes,
            "children": children,
            "w_t": w_t,
            "w_r": w_r,
            "w_l": w_l,
            "b_conv": b_conv,
        }
    )
    return out
